# revision 17
# baseline (speedup 1.0000x reference)
"""Trainium2 Bass kernel for nn_Attention2d (sparse_attention).

Reference computation (per batch b=1):
    qkv = pair_act @ W_qkv.T + b_qkv              # [S,R,3D]
    q,k,v split, reshaped to heads [S,R,H,HD]
    logits[s,h,i,j] = q[s,i,h,:] . k[s,j,h,:]
    logits masked with attention_mask[s,j] -> -1e9
    attn = softmax_j(logits) * bias[h,i,j]
    o[s,i,:] = sum_j attn . v  -> out = o @ W_out.T + b_out

Sharding: data-parallel over S (32 rows -> 4 per core, 8 cores). Each core
computes its 4 rows fully (all heads); no collectives.

Per-core layout ("transposed attention"): keys j live on SBUF partitions so
  - logitsT[j,i] accumulates in PSUM straight from the PE,
  - exp() is a single ACT pass reading PSUM,
  - softmax denominators come from a masked-ones matmul (sum over partitions),
  - the o-matmul consumes P^T directly (contraction dim on partitions).
The mask is applied by zeroing masked rows of v and of the ones-vector
(exactly equivalent to the -1e9 bias: exp(-1e9) == 0 in fp32).
The [384,384] per-head bias is DMA-transposed (bf16 xbar transpose).

All fp32 inputs are packed into ONE flat tensor host-side so that a single
SWDGE dram->dram cast feeds every DMA-transpose: the DmaTransposeAnt ISA
struct has a single sync-wait slot, so each transpose may depend on at most
one unobserved semaphore lane.  A dummy transpose absorbs the xbar-mode
transition guard; dram->dram absorber DMAs re-absorb it before the output
stores.
"""

import numpy as np

import concourse.bass as bass
import concourse.tile as tile
import concourse.mybir as mybir
from concourse import bacc
from concourse.bass_utils import run_bass_kernel_spmd
from concourse.tile_rust import add_dep_helper

# Problem shapes (hardcoded per contract; kernel.py must be self-contained).
B, S, R, D = 1, 32, 384, 256
H, HD = 8, 32
NCORES = 8
SS = S // NCORES          # 4 sequence rows per core
M = SS * R                # 1536 flattened rows per core
MT = M // 128             # 12 m-tiles
JT = R // 128             # 3 j-tiles per row
KT = D // 128             # 2 k-tiles of the model dim
F32 = mybir.dt.float32
BF16 = mybir.dt.bfloat16
AF = mybir.ActivationFunctionType
ALU = mybir.AluOpType

# Flat packing offsets (fp32 elements) of the single input tensor.
OFF_X = 0                              # [M, D]
OFF_BIAS = OFF_X + M * D               # [H*R, R]
OFF_WQKV = OFF_BIAS + H * R * R        # [3D, D]
OFF_BQKV = OFF_WQKV + 3 * D * D        # [3D]
OFF_WOUT = OFF_BQKV + 3 * D            # [D, D]
OFF_BOUT = OFF_WOUT + D * D            # [D]
OFF_M01 = OFF_BOUT + D                 # [SS, R] mask01 (1.0 = keep)
NTOT = OFF_M01 + SS * R


def build_program() -> bass.Bass:
    nc = bacc.Bacc("TRN2", target_bir_lowering=False, debug=False,
                   num_devices=NCORES)
    allin = nc.dram_tensor("allin", [NTOT], F32, kind="ExternalInput")
    out_dram = nc.dram_tensor("out", [M, D], F32, kind="ExternalOutput")
    with tile.TileContext(nc) as tc:
        _emit(nc, tc, allin, out_dram)
    nc.compile()
    return nc


def _ord(bi, deps):
    for d in deps:
        add_dep_helper(bi.ins, d.ins, sync=False, reason="dma wait budgeting")
    return bi


def _emit(nc, tc, allin, out_dram):
    from contextlib import ExitStack
    ctx = ExitStack()
    with ctx:
        singles = ctx.enter_context(tc.tile_pool(name="singles", bufs=1))
        dram = ctx.enter_context(tc.tile_pool(name="dram", bufs=1,
                                              space="DRAM"))

        # ---- Phase 0: loads, per-section bf16 casts, transposed reads ----
        # Casts are split per section so the 4.7MB bias prep stays off the
        # critical path (x/w feed phase 1; bias is first needed mid-phase 2).
        # Bacc's nop-fusion legalizes multi-wait instructions, so no manual
        # wait budgeting is needed.
        bq_sb = singles.tile([128, 4], F32)
        bv_f32 = singles.tile([1, D], F32)
        bo_f32 = singles.tile([1, D], F32)
        mb01 = singles.tile([128, SS, JT], F32)
        nc.sync.dma_start(
            out=bq_sb[:],
            in_=allin[OFF_BQKV:OFF_BQKV + 512]
            .rearrange("(t p) -> p t", p=128))
        nc.sync.dma_start(
            out=bv_f32[:],
            in_=allin[OFF_BQKV + 2 * D:OFF_BQKV + 3 * D]
            .rearrange("(a b) -> a b", a=1))
        nc.sync.dma_start(
            out=bo_f32[:],
            in_=allin[OFF_BOUT:OFF_BOUT + D]
            .rearrange("(a b) -> a b", a=1))
        nc.sync.dma_start(
            out=mb01[:],
            in_=allin[OFF_M01:OFF_M01 + SS * R]
            .rearrange("(s t p) -> p s t", p=128, t=JT))

        allbf = dram.tile([NTOT], BF16)
        xbf = allbf[OFF_X:OFF_BIAS].rearrange("(r c) -> r c", c=D)
        biasbf = allbf[OFF_BIAS:OFF_WQKV].rearrange("(r c) -> r c", c=R)
        wqbf = allbf[OFF_WQKV:OFF_BQKV].rearrange("(r c) -> r c", c=D)
        wobf = allbf[OFF_WOUT:OFF_BOUT].rearrange("(r c) -> r c", c=D)

        # x section first (feeds everything), then weights, then bias.
        nc.gpsimd.dma_start(out=allbf[OFF_X:OFF_BIAS],
                            in_=allin[OFF_X:OFF_BIAS])
        xT = singles.tile([128, KT, M], BF16)
        for kt in range(KT):
            nc.sync.dma_start(out=xT[:, kt, :],
                              in_=xbf[:, kt * 128:(kt + 1) * 128],
                              transpose=True)
        nc.gpsimd.dma_start(out=allbf[OFF_WQKV:OFF_BOUT + D],
                            in_=allin[OFF_WQKV:OFF_BOUT + D])
        wqT = singles.tile([128, KT, 3 * D], BF16)
        for kt in range(KT):
            nc.sync.dma_start(out=wqT[:, kt, :],
                              in_=wqbf[:, kt * 128:(kt + 1) * 128],
                              transpose=True)
        woT = singles.tile([128, KT, D], BF16)
        for kt in range(KT):
            nc.sync.dma_start(out=woT[:, kt, :],
                              in_=wobf[:, kt * 128:(kt + 1) * 128],
                              transpose=True)
        nc.gpsimd.dma_start(out=allbf[OFF_BIAS:OFF_WQKV],
                            in_=allin[OFF_BIAS:OFF_WQKV])
        biasT = singles.tile([128, JT, H * R], BF16)
        for jt in range(JT):
            nc.sync.dma_start(out=biasT[:, jt, :],
                              in_=biasbf[:, jt * 128:(jt + 1) * 128],
                              transpose=True)

        # small bf16 helper tiles
        bv_bf = singles.tile([1, D], BF16)
        nc.vector.tensor_copy(bv_bf[:], bv_f32[:])
        bo_bf = singles.tile([1, D], BF16)
        nc.vector.tensor_copy(bo_bf[:], bo_f32[:])
        ones32 = singles.tile([128, 32], BF16)
        nc.vector.memset(ones32[:], 1.0)
        ones_k1 = singles.tile([1, 128], BF16)
        nc.vector.memset(ones_k1[:], 1.0)
        # mask01 replicated over 32 columns, bf16 (ones-matmul stationary op)
        m01rep = singles.tile([128, SS, JT, 32], BF16)
        for s in range(SS):
            for jt in range(JT):
                nc.vector.tensor_scalar_mul(m01rep[:, s, jt, :], ones32[:],
                                            mb01[:, s, jt:jt + 1])

        # ---- Phase 1: qkv projection ----
        # qkT[n, m] for n in q(0:256)|k(256:512): 4 n-tiles
        qkT = singles.tile([128, 4, M], BF16)
        # v[m, d] natural layout
        vsb = singles.tile([128, MT, D], BF16)
        with tc.tile_pool(name="ps_qk", bufs=4, space="PSUM") as ps_qk, \
             tc.tile_pool(name="ps_v", bufs=2, space="PSUM") as ps_v:
            for nt in range(4):
                for mc in range(3):  # m in chunks of 512
                    pqk = ps_qk.tile([128, 512], F32)
                    for kt in range(KT):
                        nc.tensor.matmul(
                            pqk[:],
                            wqT[:, kt, nt * 128:(nt + 1) * 128],
                            xT[:, kt, mc * 512:(mc + 1) * 512],
                            start=(kt == 0), stop=(kt == KT - 1))
                    dst = qkT[:, nt, mc * 512:(mc + 1) * 512]
                    if (nt + mc) % 2 == 0:
                        nc.vector.tensor_scalar_add(dst, pqk[:],
                                                    bq_sb[:, nt:nt + 1])
                    else:
                        nc.scalar.activation(dst, pqk[:], AF.Identity,
                                             bias=bq_sb[:, nt:nt + 1])
            for mt in range(MT):
                pv = ps_v.tile([128, D], F32)
                # bias broadcast preload (b_v is all-zero in practice)
                nc.tensor.matmul(pv[:], ones_k1[:], bv_bf[:],
                                 start=True, stop=False)
                for kt in range(KT):
                    nc.tensor.matmul(
                        pv[:],
                        xT[:, kt, mt * 128:(mt + 1) * 128],
                        wqT[:, kt, 2 * D:3 * D],
                        start=False, stop=(kt == KT - 1))
                if mt % 2 == 0:
                    nc.vector.tensor_copy(vsb[:, mt, :], pv[:])
                else:
                    nc.scalar.copy(vsb[:, mt, :], pv[:])

        # ---- Phase 2: attention per (s, head-group) ----
        # oT[d, (s,i)]: normalized attention output, transposed for out-proj
        oT = singles.tile([128, KT, M], BF16)
        work = ctx.enter_context(tc.tile_pool(name="work", bufs=2))
        pt_pool = ctx.enter_context(tc.tile_pool(name="pt", bufs=6))
        pbt_pool = ctx.enter_context(tc.tile_pool(name="pbt", bufs=4))
        rec_pool = ctx.enter_context(tc.tile_pool(name="rec", bufs=2))
        with tc.tile_pool(name="ps_lg", bufs=2, space="PSUM") as ps_lg, \
             tc.tile_pool(name="ps_den", bufs=1, space="PSUM") as ps_den, \
             tc.tile_pool(name="ps_o", bufs=1, space="PSUM") as ps_o:
            for s in range(SS):
                # masked v for this row: zero out masked j rows
                vmask = work.tile([128, JT, D], BF16)
                for jt in range(JT):
                    nc.vector.tensor_scalar_mul(
                        vmask[:, jt, :], vsb[:, s * JT + jt, :],
                        mb01[:, s, jt:jt + 1])
                for g in range(2):  # head groups of 4
                    pts = []
                    pbts = []
                    for hp in range(4):
                        h = 4 * g + hp
                        # logitsT[j, i] = k^T . q ; 512-strided PSUM banks
                        lg = ps_lg.tile([128, JT, 512], F32)
                        for jt in range(JT):
                            nc.tensor.matmul(
                                lg[:, jt, 0:R],
                                qkT[32 * hp:32 * hp + 32, 2 + g,
                                    s * R + jt * 128: s * R + (jt + 1) * 128],
                                qkT[32 * hp:32 * hp + 32, g,
                                    s * R:(s + 1) * R],
                                start=True, stop=True,
                                tile_position=(32 * hp, 0))
                        # P^T = exp(logitsT), one ACT pass over strided banks
                        pt = pt_pool.tile([128, JT, R], BF16)
                        nc.scalar.activation(pt[:], lg[:, :, 0:R], AF.Exp)
                        pts.append(pt)
                        # biased attention weights: P^T * bias^T (bf16 2x DVE)
                        pbt = pbt_pool.tile([128, JT, R], BF16)
                        nc.vector.tensor_mul(
                            pbt[:], pt[:],
                            biasT[:, :, h * R:(h + 1) * R])
                        pbts.append(pbt)
                    # softmax denominators: masked-ones matmul, 4 heads
                    # col-packed into one bank, each head's sum replicated
                    # over its 32-partition block.
                    den = ps_den.tile([128, R], F32)
                    for hp in range(4):
                        for jt in range(JT):
                            nc.tensor.matmul(
                                den[32 * hp:32 * hp + 32, :],
                                m01rep[:, s, jt, :],
                                pts[hp][:, jt, :],
                                start=(jt == 0), stop=(jt == JT - 1),
                                tile_position=(0, 32 * hp))
                    # o^T[d, i] for the 4 heads, col-packed into one bank
                    po = ps_o.tile([128, R], F32)
                    for hp in range(4):
                        h = 4 * g + hp
                        for jt in range(JT):
                            nc.tensor.matmul(
                                po[32 * hp:32 * hp + 32, :],
                                vmask[:, jt, h * HD:(h + 1) * HD],
                                pbts[hp][:, jt, :],
                                start=(jt == 0), stop=(jt == JT - 1),
                                tile_position=(0, 32 * hp))
                    rec = rec_pool.tile([128, R], F32)
                    nc.vector.reciprocal(rec[:], den[:])
                    nc.vector.tensor_mul(oT[:, g, s * R:(s + 1) * R],
                                         po[:], rec[:])

        # ---- Phase 3: output projection ----
        with tc.tile_pool(name="ps_out", bufs=2, space="PSUM") as ps_out, \
             tc.tile_pool(name="outf", bufs=3) as outf:
            for mt in range(MT):
                pf = ps_out.tile([128, D], F32)
                nc.tensor.matmul(pf[:], ones_k1[:], bo_bf[:],
                                 start=True, stop=False)
                for kt in range(KT):
                    nc.tensor.matmul(
                        pf[:],
                        oT[:, kt, mt * 128:(mt + 1) * 128],
                        woT[:, kt, :],
                        start=False, stop=(kt == KT - 1))
                fo = outf.tile([128, D], F32)
                if mt % 2 == 0:
                    nc.vector.tensor_copy(fo[:], pf[:])
                else:
                    nc.scalar.copy(fo[:], pf[:])
                nc.sync.dma_start(
                    out=out_dram[mt * 128:(mt + 1) * 128, :], in_=fo[:])


def make_in_maps(pair_act, attention_mask, bias, W_qkv, b_qkv, W_out, b_out):
    """Shard the full inputs across the 8 cores (data-parallel over S)."""
    pair_act = np.asarray(pair_act, dtype=np.float32)
    mask01 = 1.0 - np.asarray(attention_mask).astype(np.float32)  # 1 = keep
    bias = np.asarray(bias, dtype=np.float32).reshape(H * R, R)
    W_qkv = np.asarray(W_qkv, dtype=np.float32)
    b_qkv = np.asarray(b_qkv, dtype=np.float32)
    W_out = np.asarray(W_out, dtype=np.float32)
    b_out = np.asarray(b_out, dtype=np.float32)
    shared = np.concatenate([
        bias.ravel(), W_qkv.ravel(), b_qkv.ravel(), W_out.ravel(),
        b_out.ravel()])
    in_maps = []
    for c in range(NCORES):
        sl = slice(c * SS, (c + 1) * SS)
        allin = np.concatenate([
            pair_act[0, sl].ravel(), shared, mask01[0, sl].ravel()])
        assert allin.size == NTOT
        in_maps.append({"allin": np.ascontiguousarray(
            allin.astype(np.float32))})
    return in_maps


_PROGRAM_CACHE = {}


def kernel(pair_act, attention_mask, bias, W_qkv, b_qkv, W_out, b_out,
           _want_results=False, **extra):
    in_maps = make_in_maps(pair_act, attention_mask, bias, W_qkv, b_qkv,
                           W_out, b_out)
    if "nc" not in _PROGRAM_CACHE:
        _PROGRAM_CACHE["nc"] = build_program()
    nc = _PROGRAM_CACHE["nc"]
    res = run_bass_kernel_spmd(nc, in_maps, core_ids=list(range(NCORES)))
    out = np.concatenate(
        [r["out"].reshape(SS, R, D) for r in res.results], axis=0)
    out = out.reshape(B, S, R, D).astype(np.float32)
    if _want_results:
        return out, res
    return out
